# revision 43
# baseline (speedup 1.0000x reference)
"""Trainium2 Bass kernel for nn_LongRangeDW (dense_cnn).

The module is entirely linear in x:
  s = nnstacking(x)                        (5 shifted copies, clipped to window)
  y = dw1(s) + dw2(s) + dw3(s)             (depthwise 1x1 + 3x3 d8 + 3x3 d12)
  out = pw(y) + x                          (pointwise 5C->C + residual)

Folding the depthwise taps into the pointwise gives, per nnstacking group g
with shift sigma_g and tap tau:
  out[o, p] = sum_{g,t} (W4_g diag(k_{g,t}))[o,:] @ xe[:, p + tau_t + sigma_g]
              + beff[o] + x[o, p]
with xe = zero-extended x: 85 distinct offsets. The non-offloaded offsets run
as fp8 DoubleRow matmul PAIRS on the tensor engine: two 128x128 fp8 matrices
(scaled by 2^12 into e4m3 range) stream two shifted image views together at
2 column-pairs/cycle -- half the bf16 cost per term. The pair's second view
is expressed directly as an AP [K, 2, rows, W] whose dim-1 stride is the
offset delta into the padded fp8 image.

21 taps (group 4's 17 + 4 of group 0's) are offloaded to the Vector+Scalar
engines as per-channel-scalar FMAs on a bf16 copy of the image: 9 multiplies
run in the DVE 4x perf mode on fully contiguous padded-width spans (|dx| <=
PAD keeps row-wrap garbage inside the pad columns), 12 on the Scalar engine
(activation Copy with per-partition scale, center span only -- ACT has its
own SBUF port so it never contends with the DVE, unlike GpSimd which shares
the DVE's second port under an exclusive full-instruction lock).
Accumulation is DVE 2x tensor_tensor on the center views.  Each offloaded
group's y feeds one bf16 pointwise matmul per sub-block.  Offloading work
off the PE also eased the chip power throttle (util limit 0.91 -> ~1.0),
speeding every remaining matmul ~17%.

The tap pipeline (not the PE) gates the kernel tail, so: y tiles rotate 6
deep (the y-buffer WAR release paces when a granule can start), the bf16
image + per-tap scalars are the first DMAs issued, and for the LAST granule
most taps run on the PE instead (extra fp8 pairs) so the final granule is
short.  Evacuation is a single DVE dual-op tensor_scalar
(out = (psum + 4096*beff) * 2^-12); psum rotates over all 8 banks (4 tiles).
The boundary-correction matmuls read a tiny host-packed border tensor so
they need not wait for the full image DMA.

Boundary exactness: composing clipped shifts with zero-padded convs is NOT the
padded composite. Where a depthwise tap lands exactly 1 px outside the window
and sigma_g pulls it back in, the composite wrongly reads x. The mismatch
lives on 8 one-pixel strips (output rows/cols {7,11,116,120}) reading x's 4
border lines -> 24 small correction matmuls folded in during evacuation.

The residual enters PSUM as an ident*2^12 matmul of the bf16 image (0.4% of
|x|, inside the error budget; saves the 8.4MB fp32 x transfer).

Measured on trn2: ~291us (from the 344us two-engine version and 477us for
the all-bf16 single-engine version).

Data parallel: batch B=8 -> one image per NeuronCore.
"""

import sys

import numpy as np

sys.path.insert(0, "/opt/trn_rl_repo")

B, C, H, W = 8, 128, 128, 128
PAD = 14            # max |offset| = 13, rounded even for DVE 4B alignment
HP = H + 2 * PAD
WP = W + 2 * PAD
N_CORES = 8
SB_ROWS = 8         # output rows per super-block (psum tile = 2 banks)
N_SB = H // SB_ROWS
SUB_ROWS = 4        # rows per matmul (out free dim 512 = one PSUM bank)

WSCALE = 4096.0     # fp8 weight scale (2^12); removed at evacuation

SHIFTS = [(1, 0), (-1, 0), (0, 1), (0, -1), (0, 0)]  # nnstacking groups

# (group, n_taps, act_mul_tap_indices): taps offloaded off the PE.  All
# multiplies whose tap index is in the act set run on the Scalar engine
# (activation Copy with per-partition scale, dedicated SBUF port); the rest
# run on the Vector engine in 4x mode (needs even dx -> groups 0, 1, 4).
# Accumulation is always Vector tensor_tensor (2x mode).  GpSimd compute is
# OFF-LIMITS: it arbitrates for the same shared SBUF port pair the DVE perf
# modes need (exclusive full-instruction lock) -- the v4 experiment that put
# adds there dropped DVE to 2x and slowed the kernel to 477us.
OFFLOAD = [(4, 17, frozenset({1, 3, 5, 7, 9, 11, 13, 15})),
           (0, 4, frozenset({0, 1, 2, 3}))]
# Per offloaded group: tap indices that run on the PE (as extra fp8 pairs)
# for the LAST tap granule only: shrinks the final granule's DVE/ACT
# latency, which otherwise leaves the PE idling ~11us at the end of the
# pipeline (no pair-matmul work left to overlap the last granule's taps).
# The total tap count across groups must be even (fp8 pairs).
TAIL_PE_TAPS = {0: tuple(range(4)),
                4: (0, 2, 4, 6, 8, 10, 12, 14)}


# --------------------------------------------------------------------------
# host-side operator folding
# --------------------------------------------------------------------------

def _group_taps(w1, w2, w3, g):
    """All 17 taps of group g as {(di, dj): kvec[C]} (shift folded in)."""
    sy, sx = SHIFTS[g]
    sl = slice(g * C, (g + 1) * C)
    taps = {}

    def add(di, dj, kv):
        v = taps.setdefault((di, dj), np.zeros(C, np.float64))
        v += kv.astype(np.float64)

    add(sy, sx, w1[sl, 0, 0, 0])
    for w, d in ((w2, 8), (w3, 12)):
        for a in range(3):
            for b in range(3):
                add(sy + (a - 1) * d, sx + (b - 1) * d, w[sl, 0, a, b])
    return taps


def _build_terms(w1, w2, w3, w4):
    """Returns (offsets, mats, off_specs, extra_offsets, extra_mats).
    off_specs is a list per OFFLOAD entry: dict(g, tap_offsets, kmat [C, n],
    act_set, w4g [C, C]).  extra_* are per-tap fold matrices for the
    TAIL_PE_GROUPS offloaded taps (used only for the last granule's SBs)."""
    w4m = w4[:, :, 0, 0].astype(np.float64)  # [C, 5C]
    offload_n = {g: (n, act) for g, n, act in OFFLOAD}
    mat_terms = {}
    off_specs = []
    extra_offsets, extra_mats = [], []
    for g in range(5):
        taps = _group_taps(w1, w2, w3, g)
        tap_offsets = sorted(taps)
        n_off, act_set = offload_n.get(g, (0, frozenset()))
        off, keep = tap_offsets[:n_off], tap_offsets[n_off:]
        if off:
            kmat = np.stack([taps[o] for o in off], axis=1)  # [C, n]
            tail_pe = tuple(t for t in TAIL_PE_TAPS.get(g, ()) if t < len(off))
            off_specs.append(dict(
                g=g, tap_offsets=off, kmat=kmat.astype(np.float32),
                act_set=act_set, tail_pe=frozenset(tail_pe),
                w4g=w4m[:, g * C:(g + 1) * C].astype(np.float32)))
            for t in tail_pe:
                o = off[t]
                extra_offsets.append(o)
                extra_mats.append(
                    w4m[:, g * C:(g + 1) * C] * taps[o][None, :])
        for o in keep:
            M = mat_terms.setdefault(o, np.zeros((C, C), np.float64))
            M += w4m[:, g * C:(g + 1) * C] * taps[o][None, :]
    offsets = sorted(mat_terms)
    mats = np.stack([mat_terms[o] for o in offsets]).astype(np.float32)
    extra = (np.stack(extra_mats).astype(np.float32) if extra_mats
             else np.zeros((0, C, C), np.float32))
    return offsets, mats, off_specs, extra_offsets, extra


def _build_corrections(w2, w3, w4):
    """24 strip-correction terms (matrices already NEGATED for accumulation).

    Strips j<4: column strips (out col px, read x col src, row shift ty);
    j>=4: row strips. Each strip has 3 taps."""
    w4m = w4[:, :, 0, 0].astype(np.float64)
    strips, mats = [], []
    specs = [
        ("col", 2, 8), ("col", 2, 12), ("col", 3, 12), ("col", 3, 8),
        ("row", 0, 8), ("row", 0, 12), ("row", 1, 12), ("row", 1, 8),
    ]
    for kind, g, d in specs:
        sy, sx = SHIFTS[g]
        sl = slice(g * C, (g + 1) * C)
        w = w2 if d == 8 else w3
        if kind == "col":
            border = -1 if sx == 1 else W
            fixed_out = border - (-d if sx == 1 else d)
            src = border + sx
            shifts = [-d, 0, d]                     # ty values
            tap_b = 0 if sx == 1 else 2
            kvs = [w[sl, 0, a, tap_b] for a in range(3)]
        else:
            border = -1 if sy == 1 else H
            fixed_out = border - (-d if sy == 1 else d)
            src = border + sy
            shifts = [-d, 0, d]                     # tx values
            tap_a = 0 if sy == 1 else 2
            kvs = [w[sl, 0, tap_a, b] for b in range(3)]
        strips.append(dict(kind=kind, fixed_out=fixed_out, src=src, shifts=shifts))
        for kv in kvs:
            mats.append(-(w4m[:, sl] * kv.astype(np.float64)[None, :]))
    return strips, np.stack(mats).astype(np.float32)


def _build_weights(inputs):
    w1, w2, w3, w4 = inputs["w1"], inputs["w2"], inputs["w3"], inputs["w4"]
    b1, b2, b3, b4 = inputs["b1"], inputs["b2"], inputs["b3"], inputs["b4"]
    offsets, mats, off_specs, extra_offsets, extra_mats = _build_terms(
        w1, w2, w3, w4)
    strips, cmats = _build_corrections(w2, w3, w4)
    # fp8 stationary: per-offset fold matrices, scaled into e4m3 range.
    # Saturate at the ml_dtypes float8_e4m3 max (240): values beyond it
    # would cast to inf. No-op for the reference weights (absmax 188).
    # The tail-granule extra matrices are appended after the main terms.
    all_mats = np.concatenate([mats, extra_mats], axis=0)
    wt8 = np.ascontiguousarray(
        np.clip((all_mats * WSCALE), -240.0, 240.0)
        .transpose(2, 0, 1).reshape(C, -1))                      # [C, T*C]
    # bf16 stationary: 24 corrections (scaled) + per-group PW (scaled) +
    # ident (strip folds, x1) + ident*WSCALE (residual-into-psum)
    pw = np.stack([sp["w4g"] for sp in off_specs])               # [n_off, C, C]
    ident = np.eye(C, dtype=np.float32)[None]
    wtb = np.concatenate([cmats * WSCALE, pw * WSCALE, ident,
                          ident * WSCALE], axis=0)
    wtb = np.ascontiguousarray(wtb.transpose(2, 0, 1).reshape(C, -1))
    ks = np.concatenate([sp["kmat"] for sp in off_specs], axis=1)  # [C, ntaps]
    w4m = w4[:, :, 0, 0].astype(np.float64)
    beff = (b4.astype(np.float64)
            + w4m @ (b1 + b2 + b3).astype(np.float64)).astype(np.float32)
    return wt8, wtb, ks, beff, offsets, off_specs, strips, extra_offsets


# --------------------------------------------------------------------------
# device program
# --------------------------------------------------------------------------

_CACHE = {}


def _build_program(offsets, off_specs, strips, extra_offsets):
    import concourse.bacc as bacc
    import concourse.mybir as mybir
    import concourse.tile as tile
    from concourse.ap import AP

    nc = bacc.Bacc("TRN2", target_bir_lowering=False)
    f32 = mybir.dt.float32
    bf16 = mybir.dt.bfloat16
    f8 = mybir.dt.float8e4

    n_terms = len(offsets)
    n_pairs = n_terms // 2
    n_single = n_terms % 2
    n_extra = len(extra_offsets)
    assert n_extra % 2 == 0, "tail-PE taps must pair up"
    n_t8 = n_terms + n_extra
    n_off = len(off_specs)
    n_ks = sum(len(sp["tap_offsets"]) for sp in off_specs)
    # bf16 block indices
    CORR_BLK = 0
    PW_BLK = 24
    ID_BLK = 24 + n_off
    RES_BLK = 24 + n_off + 1
    nb_blk = 24 + n_off + 2

    xp8_d = nc.dram_tensor("xp8", [C, HP * WP], f8, kind="ExternalInput")
    xpb_d = nc.dram_tensor("xpb", [C, HP * WP], bf16, kind="ExternalInput")
    # the 4 border lines the strip corrections read (2 padded cols + 2
    # padded rows), packed so the corrections don't wait on the full image
    xbrd_d = nc.dram_tensor("xbrd", [C, 2 * HP + 2 * WP], bf16,
                            kind="ExternalInput")
    wt8_d = nc.dram_tensor("wt8", [C, n_t8 * C], f8, kind="ExternalInput")
    wtb_d = nc.dram_tensor("wtb", [C, nb_blk * C], bf16, kind="ExternalInput")
    ks_d = nc.dram_tensor("ks", [C, n_ks], f32, kind="ExternalInput")
    beff_d = nc.dram_tensor("beff", [C, 1], f32, kind="ExternalInput")
    out_d = nc.dram_tensor("out", [C, H * W], f32, kind="ExternalOutput")

    with tile.TileContext(nc) as tc:
        with (
            tc.tile_pool(name="const", bufs=1) as const,
            tc.tile_pool(name="outp", bufs=2) as outp,
            tc.tile_pool(name="yp", bufs=3) as yp,
            tc.tile_pool(name="up", bufs=2) as up,
            tc.tile_pool(name="psum", bufs=4, space="PSUM") as psum_pool,
        ):
            xp8_sb = const.tile([C, HP * WP], f8)
            xpb_sb = const.tile([C, HP * WP], bf16)
            xbrd_sb = const.tile([C, 2 * HP + 2 * WP], bf16)
            wt8_sb = const.tile([C, n_t8 * C], f8)
            wtb_sb = const.tile([C, nb_blk * C], bf16)
            ks_sb = const.tile([C, n_ks], f32)
            beff_sb = const.tile([C, 1], f32)

            # SWDGE (nc.gpsimd) fans >=1MB transfers across all 16 SDMA
            # engines (~340 GB/s); HWDGE runs ~26 GB/s on a single engine.
            # Order: minimum needed for SB0 first, then all of xpb (the
            # SB0-time corrections read the whole bf16 image), then xp8.
            # DMA order = the two critical chains' first needs, interleaved:
            # PE's first matmuls need the first weight pairs + SB0's fp8
            # rows; the DVE/ACT tap queue needs xpb's first granule + ks.
            nc.sync.dma_start(out=ks_sb, in_=ks_d[:, :])
            nc.gpsimd.dma_start(out=xbrd_sb, in_=xbrd_d[:, :])
            W8SPLIT = min(16 * C, n_t8 * C)
            nc.gpsimd.dma_start(out=wt8_sb[:, :W8SPLIT],
                                in_=wt8_d[:, :W8SPLIT])
            ROWS0 = SB_ROWS + 2 * PAD
            nc.gpsimd.dma_start(out=xp8_sb[:, :ROWS0 * WP],
                                in_=xp8_d[:, :ROWS0 * WP])
            nc.gpsimd.dma_start(out=xpb_sb[:, :(2 * SB_ROWS + 2 * PAD) * WP],
                                in_=xpb_d[:, :(2 * SB_ROWS + 2 * PAD) * WP])
            nc.gpsimd.dma_start(out=wt8_sb[:, W8SPLIT:],
                                in_=wt8_d[:, W8SPLIT:])
            nc.gpsimd.dma_start(out=wtb_sb, in_=wtb_d[:, :])
            XP_CHUNK_ROWS = 48
            for r0_ in range(2 * SB_ROWS + 2 * PAD, HP, XP_CHUNK_ROWS):
                r1_ = min(r0_ + XP_CHUNK_ROWS, HP)
                nc.gpsimd.dma_start(out=xpb_sb[:, r0_ * WP:r1_ * WP],
                                    in_=xpb_d[:, r0_ * WP:r1_ * WP])
            for r0_ in range(ROWS0, HP, XP_CHUNK_ROWS):
                r1_ = min(r0_ + XP_CHUNK_ROWS, HP)
                nc.gpsimd.dma_start(out=xp8_sb[:, r0_ * WP:r1_ * WP],
                                    in_=xp8_d[:, r0_ * WP:r1_ * WP])
            nc.sync.dma_start(out=beff_sb, in_=beff_d[:, :])

            xp3 = xp8_sb.rearrange("p (r w) -> p r w", w=WP)

            def wblk8_pair(p):
                return wt8_sb[:, 2 * p * C:(2 * p + 2) * C].rearrange(
                    "p (two m) -> p two m", two=2)

            def wblk8(i):
                return wt8_sb[:, i * C:(i + 1) * C]

            def wblkb(i):
                return wtb_sb[:, i * C:(i + 1) * C]

            def pair_rhs(o_a, o_b, a0, sub):
                """rhs AP [C, 2, sub, W]: two shifted views, pair delta."""
                va = xp3[:, a0 + o_a[0]: a0 + o_a[0] + sub,
                         PAD + o_a[1]: PAD + o_a[1] + W]
                delta = (o_b[0] - o_a[0]) * WP + (o_b[1] - o_a[1])
                ap = list(va.ap)
                ap = [ap[0], (delta, 2), ap[1], ap[2]]
                return AP(tensor=va.tensor, offset=va.offset, ap=ap)

            corr_sb = const.tile([C, 8 * H], bf16)

            xb3 = xpb_sb.rearrange("p (r w) -> p r w", w=WP)

            def emit_corrections():
                # reads the packed border tensor (bf16 weights x bf16
                # moving; do not mix dtypes in one mm).  Borrows a main-psum
                # rotation slot (same tag+size) so all 8 PSUM banks serve
                # the pipeline.
                psum_c2 = psum_pool.tile([C, 2 * SB_ROWS * W], f32,
                                         name="psum_c", tag="acc", bufs=2)
                psum_c = psum_c2[:, :8 * H]
                for j, st in enumerate(strips):
                    if st["kind"] == "col":
                        base = 0 if st["src"] == 0 else HP
                        n = H
                    else:
                        base = 2 * HP + (0 if st["src"] == 0 else WP)
                        n = W
                    for i, sh in enumerate(st["shifts"]):
                        rhs = xbrd_sb[:, base + PAD + sh:
                                      base + PAD + sh + n]
                        nc.tensor.matmul(psum_c[:, j * H:(j + 1) * H],
                                         wblkb(CORR_BLK + 3 * j + i), rhs,
                                         start=(i == 0), stop=(i == 2))
                # ACT, not DVE: DVE is busy with taps; psum_c slot release
                # should not sit behind them
                nc.scalar.copy(corr_sb, psum_c)

            # per-OFFLOAD-group scalar column base in ks
            ks_base = []
            b = 0
            for sp in off_specs:
                ks_base.append(b)
                b += len(sp["tap_offsets"])

            # ---- main loop -------------------------------------------------
            n_sub = SB_ROWS // SUB_ROWS
            SB_PER_G = 2          # tap FMAs at 2-SB granularity; 4-SB lumps
            Y_ROWS = SB_PER_G * SB_ROWS   # regressed (coarser pipeline)

            def emit_taps(gi):
                """Tap FMAs for granule gi (SBs 2*gi, 2*gi+1). Multiplies:
                DVE 4x-mode on contiguous padded-width spans (row-wrap
                garbage stays in pad cols since |dx|<PAD) or ACT activation
                Copy with per-partition scale (own SBUF port, no DVE
                contention). Accumulation on the center views: DVE 2x
                tensor_tensor. ACT products alternate between two ua tiles
                so the ACT muls and DVE adds pipeline instead of ping-pong
                serializing on one buffer. Returns [(y3_view, pw_block)]."""
                r0 = SB_PER_G * gi * SB_ROWS
                last_gran = gi == N_SB // SB_PER_G - 1
                pair_ys = []
                for oi, sp in enumerate(off_specs):
                    g = sp["g"]
                    taps = sp["tap_offsets"]
                    act_set = sp["act_set"]
                    nd = len(taps)
                    live = [t for t in range(nd)
                            if not (last_gran and t in sp["tail_pe"])]
                    if not live:
                        continue  # all taps on the PE for the tail SBs

                    def span(t_idx):
                        dy, dx = taps[t_idx]
                        off0 = (PAD + r0 + dy) * WP + dx
                        return xpb_sb[:, off0: off0 + Y_ROWS * WP]

                    def center(t_idx):
                        dy, dx = taps[t_idx]
                        return xb3[:, PAD + r0 + dy: PAD + r0 + dy + Y_ROWS,
                                   PAD + dx: PAD + dx + W]

                    def kcol(t_idx):
                        cb = ks_base[oi] + t_idx
                        return ks_sb[:, cb:cb + 1]

                    # deep y rotation lets the tap engines start a granule as
                    # soon as possible -- the y-buffer WAR release (not DVE
                    # total work) gates the pipeline tail
                    y = yp.tile([C, Y_ROWS * WP], bf16, tag=f"y{g}",
                                bufs=6 if g == 4 else 4)
                    y3 = y.rearrange("p (r w) -> p r w", w=WP)
                    yc = y3[:, :, PAD:PAD + W]
                    u = u3 = None
                    if any(t not in act_set for t in live):
                        u = up.tile([C, Y_ROWS * WP], bf16, tag=f"u{g}",
                                    bufs=3)
                        u3 = u.rearrange("p (r w) -> p r w", w=WP)
                    uas, na = [], 0
                    if act_set:
                        # ACT products cover only the consumed 128-col
                        # center (ACT has no perf modes -- cost is element
                        # count; the DVE muls keep full contiguous spans
                        # for 4x mode)
                        for j in range(2):
                            ua = up.tile([C, Y_ROWS * W], bf16,
                                         tag=f"ua{g}{j}", bufs=2)
                            uas.append(
                                (ua, ua.rearrange("p (r w) -> p r w", w=W)))

                    t0 = live[0]
                    if t0 in act_set:
                        # center-only init: y's pad columns are never read
                        nc.scalar.activation(
                            yc, center(t0),
                            mybir.ActivationFunctionType.Copy,
                            scale=kcol(t0))
                    else:
                        nc.vector.tensor_scalar_mul(y, span(t0), kcol(t0))
                    for t in live[1:]:
                        if t in act_set:
                            flat, v3 = uas[na % 2]
                            na += 1
                            nc.scalar.activation(
                                flat, center(t),
                                mybir.ActivationFunctionType.Copy,
                                scale=kcol(t))
                            src = v3
                        else:
                            nc.vector.tensor_scalar_mul(u, span(t), kcol(t))
                            src = u3[:, :, PAD:PAD + W]
                        nc.vector.tensor_tensor(
                            yc, yc, src, mybir.AluOpType.add)
                    pair_ys.append((y3, PW_BLK + oi))
                return pair_ys

            granule_ys = {0: emit_taps(0)}
            for s in range(N_SB):
                r0 = s * SB_ROWS
                half = (s % SB_PER_G) * SB_ROWS
                ys = [(y3[:, half:half + SB_ROWS, PAD:PAD + W], blk)
                      for y3, blk in granule_ys[s // SB_PER_G]]

                # one psum tile (4 banks) per 2 SBs: halves the evac op
                # count and the output-store descriptor generations
                if s % 2 == 0:
                    psum2 = psum_pool.tile([C, 2 * SB_ROWS * W], f32,
                                           tag="acc", bufs=2)
                psum = psum2[:, (s % 2) * SB_ROWS * W:
                             (s % 2 + 1) * SB_ROWS * W]
                pair_list = [(offsets[2 * p], offsets[2 * p + 1], p)
                             for p in range(n_pairs)]
                if s // SB_PER_G == N_SB // SB_PER_G - 1:
                    pair_list += [
                        (extra_offsets[2 * j], extra_offsets[2 * j + 1],
                         n_pairs + j) for j in range(n_extra // 2)]
                for i, (o_a, o_b, p) in enumerate(pair_list):
                    for u_ in range(n_sub):
                        a0 = PAD + r0 + u_ * SUB_ROWS
                        nc.tensor.matmul(
                            psum[:, u_ * SUB_ROWS * W:(u_ + 1) * SUB_ROWS * W],
                            wblk8_pair(p), pair_rhs(o_a, o_b, a0, SUB_ROWS),
                            start=(i == 0), stop=False,
                            skip_group_check=True,
                            perf_mode=mybir.MatmulPerfMode.DoubleRow)
                if n_single:
                    di, dj = offsets[-1]
                    for u_ in range(n_sub):
                        a0 = PAD + r0 + u_ * SUB_ROWS + di
                        nc.tensor.matmul(
                            psum[:, u_ * SUB_ROWS * W:(u_ + 1) * SUB_ROWS * W],
                            wblk8(n_terms - 1),
                            xp3[:, a0: a0 + SUB_ROWS, PAD + dj: PAD + dj + W],
                            start=False, stop=False)
                for yv, blk in ys:
                    for u_ in range(n_sub):
                        nc.tensor.matmul(
                            psum[:, u_ * SUB_ROWS * W:(u_ + 1) * SUB_ROWS * W],
                            wblkb(blk),
                            yv[:, u_ * SUB_ROWS:(u_ + 1) * SUB_ROWS, :],
                            start=False, stop=False,
                            skip_group_check=True)

                if s == 0:
                    emit_corrections()
                # prefetch next granule's taps ahead of this SB's
                # evacuation so the DVE/ACT/GpSimd queues never sit behind
                # the psum drain
                if s % SB_PER_G == 0 and s // SB_PER_G + 1 < N_SB // SB_PER_G:
                    granule_ys[s // SB_PER_G + 1] = emit_taps(s // SB_PER_G + 1)

                # fold strip corrections into PSUM on the PE: identity-weight
                # matmuls add corr_sb rows into strided psum positions
                psum3 = psum.rearrange("p (r w) -> p r w", w=W)
                # all 4 col strips in ONE ident matmul: their psum columns
                # {7,11,116,120} factor as c0 + a*da + b*db with the strips
                # laid out j = 2a+b in corr_sb -> congruent 4-dim APs
                cols = [st["fixed_out"] for st in strips if st["kind"] == "col"]
                c0, db, da = cols[0], cols[1] - cols[0], cols[2] - cols[0]
                assert cols == [c0, c0 + db, c0 + da, c0 + da + db]
                src0 = corr_sb[:, r0: r0 + SB_ROWS]
                rhs4 = AP(tensor=src0.tensor, offset=src0.offset,
                          ap=[src0.ap[0], (2 * H, 2), (H, 2), (1, SB_ROWS)])
                dst0 = psum[:, c0:c0 + 1]
                out4 = AP(tensor=dst0.tensor, offset=dst0.offset,
                          ap=[dst0.ap[0], (da, 2), (db, 2), (W, SB_ROWS)])
                strip_mms = [(out4, rhs4)]
                for j, st in enumerate(strips):
                    if st["kind"] == "row" and r0 <= st["fixed_out"] < r0 + SB_ROWS:
                        lr = st["fixed_out"] - r0
                        strip_mms.append((psum3[:, lr:lr + 1, :],
                                          corr_sb[:, j * H: j * H + W]))
                for i, (dst, src) in enumerate(strip_mms):
                    nc.tensor.matmul(dst, wblkb(ID_BLK), src,
                                     start=False, stop=False,
                                     skip_group_check=True)
                # residual into PSUM: ident*WSCALE applied to the bf16 image
                # (0.4% of |x|, inside the error budget; saves the fp32 x DMA
                # and keeps the whole evacuation off the busy Vector engine)
                for u_ in range(n_sub):
                    a0 = PAD + r0 + u_ * SUB_ROWS
                    nc.tensor.matmul(
                        psum[:, u_ * SUB_ROWS * W:(u_ + 1) * SUB_ROWS * W],
                        wblkb(RES_BLK),
                        xb3[:, a0: a0 + SUB_ROWS, PAD:PAD + W],
                        start=False,
                        stop=(u_ == n_sub - 1) and s % 2 == 1,
                        skip_group_check=True)

                # single DVE dual-op per SB pair:
                # out = (psum + beff*4096) * 2^-12.  (ACT would need two
                # ops -- HW drops the scale when scale and bias are
                # combined -- and the DVE has the spare capacity now that
                # ACT carries most tap multiplies.)
                if s % 2 == 1:
                    out_sb = outp.tile([C, 2 * SB_ROWS * W], f32)
                    nc.vector.tensor_scalar(out_sb, psum2, beff_sb[:, 0:1],
                                            1.0 / WSCALE,
                                            mybir.AluOpType.add,
                                            mybir.AluOpType.mult)
                    nc.gpsimd.dma_start(
                        out=out_d[:, (r0 - SB_ROWS) * W:(r0 + SB_ROWS) * W],
                        in_=out_sb)
    nc.finalize()
    return nc


def _make_in_maps(inputs):
    x = np.ascontiguousarray(inputs["x"], dtype=np.float32)
    (wt8, wtb, ks, beff, offsets, off_specs, strips,
     extra_offsets) = _build_weights(inputs)
    if "nc" not in _CACHE:
        _CACHE["nc"] = _build_program(offsets, off_specs, strips,
                                      extra_offsets)

    import ml_dtypes
    bf = ml_dtypes.bfloat16
    f8 = ml_dtypes.float8_e4m3
    xpad8 = np.zeros((B, C, HP, WP), f8)
    xpad8[:, :, PAD:PAD + H, PAD:PAD + W] = x.astype(f8)
    xpadb = np.zeros((B, C, HP, WP), bf)
    xpadb[:, :, PAD:PAD + H, PAD:PAD + W] = x.astype(bf)
    beff_col = np.ascontiguousarray((beff * WSCALE).reshape(C, 1))
    wt8_f8 = wt8.astype(f8)
    wtb_bf = wtb.astype(bf)
    ksc = np.ascontiguousarray(ks)
    return [
        {
            "xp8": np.ascontiguousarray(xpad8[b].reshape(C, HP * WP)),
            "xpb": np.ascontiguousarray(xpadb[b].reshape(C, HP * WP)),
            "xbrd": np.ascontiguousarray(np.concatenate(
                [xpadb[b, :, :, PAD + 0], xpadb[b, :, :, PAD + W - 1],
                 xpadb[b, :, PAD + 0, :], xpadb[b, :, PAD + H - 1, :]],
                axis=1)),
            "wt8": wt8_f8,
            "wtb": wtb_bf,
            "ks": ksc,
            "beff": beff_col,
        }
        for b in range(B)
    ]


def kernel(**inputs):
    in_maps = _make_in_maps(inputs)
    from concourse.bass_utils import run_bass_kernel_spmd
    res = run_bass_kernel_spmd(_CACHE["nc"], in_maps, core_ids=list(range(N_CORES)))
    out = np.stack([res.results[b]["out"].reshape(C, H, W) for b in range(B)])
    return out.astype(np.float32)



# revision 46
# speedup vs baseline: 1.3231x; 1.3231x over previous
"""Trainium2 Bass kernel for nn_LongRangeDW (dense_cnn).

The module is entirely linear in x:
  s = nnstacking(x)                        (5 shifted copies, clipped to window)
  y = dw1(s) + dw2(s) + dw3(s)             (depthwise 1x1 + 3x3 d8 + 3x3 d12)
  out = pw(y) + x                          (pointwise 5C->C + residual)

Folding the depthwise taps into the pointwise gives, per nnstacking group g
with shift sigma_g and tap tau:
  out[o, p] = sum_{g,t} (W4_g diag(k_{g,t}))[o,:] @ xe[:, p + tau_t + sigma_g]
              + beff[o] + x[o, p]
with xe = zero-extended x: 85 distinct offsets. The non-offloaded offsets run
as fp8 DoubleRow matmul PAIRS on the tensor engine: two 128x128 fp8 matrices
(scaled by 2^12 into e4m3 range) stream two shifted image views together at
2 column-pairs/cycle -- half the bf16 cost per term. The pair's second view
is expressed directly as an AP [K, 2, rows, W] whose dim-1 stride is the
offset delta into the padded fp8 image.

21 taps (group 4's 17 + 4 of group 0's) are offloaded to the Vector+Scalar
engines as per-channel-scalar FMAs on a bf16 copy of the image: 9 multiplies
run in the DVE 4x perf mode on fully contiguous padded-width spans (|dx| <=
PAD keeps row-wrap garbage inside the pad columns), 12 on the Scalar engine
(activation Copy with per-partition scale, center span only -- ACT has its
own SBUF port so it never contends with the DVE, unlike GpSimd which shares
the DVE's second port under an exclusive full-instruction lock).
Accumulation is DVE 2x tensor_tensor on the center views.  Each offloaded
group's y feeds one bf16 pointwise matmul per sub-block.  Offloading work
off the PE also eased the chip power throttle (util limit 0.91 -> ~1.0),
speeding every remaining matmul ~17%.

The tap pipeline (not the PE) gates the kernel tail, so: y tiles rotate 6
deep (the y-buffer WAR release paces when a granule can start), the bf16
image + per-tap scalars are the first DMAs issued, and for the LAST granule
most taps run on the PE instead (extra fp8 pairs) so the final granule is
short.  Evacuation is a single DVE dual-op tensor_scalar
(out = (psum + 4096*beff) * 2^-12); psum rotates over all 8 banks (4 tiles).
The boundary-correction matmuls read a tiny host-packed border tensor so
they need not wait for the full image DMA.

Boundary exactness: composing clipped shifts with zero-padded convs is NOT the
padded composite. Where a depthwise tap lands exactly 1 px outside the window
and sigma_g pulls it back in, the composite wrongly reads x. The mismatch
lives on 8 one-pixel strips (output rows/cols {7,11,116,120}) reading x's 4
border lines -> 24 small correction matmuls folded in during evacuation.

The residual enters PSUM as an ident*2^12 matmul of the bf16 image (0.4% of
|x|, inside the error budget; saves the 8.4MB fp32 x transfer).

Measured on trn2: 287.3us (from the 344us two-engine version and 477us for
the all-bf16 single-engine version).

Data parallel: batch B=8 -> one image per NeuronCore.
"""

import sys

import numpy as np

sys.path.insert(0, "/opt/trn_rl_repo")

B, C, H, W = 8, 128, 128, 128
PAD = 14            # max |offset| = 13, rounded even for DVE 4B alignment
HP = H + 2 * PAD
WP = W + 2 * PAD
N_CORES = 8
SB_ROWS = 8         # output rows per super-block (psum tile = 2 banks)
N_SB = H // SB_ROWS
SUB_ROWS = 4        # rows per matmul (out free dim 512 = one PSUM bank)

WSCALE = 4096.0     # fp8 weight scale (2^12); removed at evacuation

SHIFTS = [(1, 0), (-1, 0), (0, 1), (0, -1), (0, 0)]  # nnstacking groups

# (group, n_taps, act_mul_tap_indices): taps offloaded off the PE.  All
# multiplies whose tap index is in the act set run on the Scalar engine
# (activation Copy with per-partition scale, dedicated SBUF port); the rest
# run on the Vector engine in 4x mode (needs even dx -> groups 0, 1, 4).
# Accumulation is always Vector tensor_tensor (2x mode).  GpSimd compute is
# OFF-LIMITS: it arbitrates for the same shared SBUF port pair the DVE perf
# modes need (exclusive full-instruction lock) -- the v4 experiment that put
# adds there dropped DVE to 2x and slowed the kernel to 477us.
OFFLOAD = [(4, 17, frozenset({1, 3, 5, 7, 9, 11, 13, 15})),
           (0, 4, frozenset({0, 1, 2, 3}))]
# Per offloaded group: tap indices that run on the PE (as extra fp8 pairs)
# for the LAST tap granule only: shrinks the final granule's DVE/ACT
# latency, which otherwise leaves the PE idling ~11us at the end of the
# pipeline (no pair-matmul work left to overlap the last granule's taps).
# The total tap count across groups must be even (fp8 pairs).
TAIL_PE_TAPS = {0: tuple(range(4)),
                4: (0, 2, 4, 6, 8, 10, 12, 14)}


# --------------------------------------------------------------------------
# host-side operator folding
# --------------------------------------------------------------------------

def _group_taps(w1, w2, w3, g):
    """All 17 taps of group g as {(di, dj): kvec[C]} (shift folded in)."""
    sy, sx = SHIFTS[g]
    sl = slice(g * C, (g + 1) * C)
    taps = {}

    def add(di, dj, kv):
        v = taps.setdefault((di, dj), np.zeros(C, np.float64))
        v += kv.astype(np.float64)

    add(sy, sx, w1[sl, 0, 0, 0])
    for w, d in ((w2, 8), (w3, 12)):
        for a in range(3):
            for b in range(3):
                add(sy + (a - 1) * d, sx + (b - 1) * d, w[sl, 0, a, b])
    return taps


def _build_terms(w1, w2, w3, w4):
    """Returns (offsets, mats, off_specs, extra_offsets, extra_mats).
    off_specs is a list per OFFLOAD entry: dict(g, tap_offsets, kmat [C, n],
    act_set, w4g [C, C]).  extra_* are per-tap fold matrices for the
    TAIL_PE_GROUPS offloaded taps (used only for the last granule's SBs)."""
    w4m = w4[:, :, 0, 0].astype(np.float64)  # [C, 5C]
    offload_n = {g: (n, act) for g, n, act in OFFLOAD}
    mat_terms = {}
    off_specs = []
    extra_offsets, extra_mats = [], []
    for g in range(5):
        taps = _group_taps(w1, w2, w3, g)
        tap_offsets = sorted(taps)
        n_off, act_set = offload_n.get(g, (0, frozenset()))
        off, keep = tap_offsets[:n_off], tap_offsets[n_off:]
        if off:
            kmat = np.stack([taps[o] for o in off], axis=1)  # [C, n]
            tail_pe = tuple(t for t in TAIL_PE_TAPS.get(g, ()) if t < len(off))
            off_specs.append(dict(
                g=g, tap_offsets=off, kmat=kmat.astype(np.float32),
                act_set=act_set, tail_pe=frozenset(tail_pe),
                w4g=w4m[:, g * C:(g + 1) * C].astype(np.float32)))
            for t in tail_pe:
                o = off[t]
                extra_offsets.append(o)
                extra_mats.append(
                    w4m[:, g * C:(g + 1) * C] * taps[o][None, :])
        for o in keep:
            M = mat_terms.setdefault(o, np.zeros((C, C), np.float64))
            M += w4m[:, g * C:(g + 1) * C] * taps[o][None, :]
    offsets = sorted(mat_terms)
    mats = np.stack([mat_terms[o] for o in offsets]).astype(np.float32)
    extra = (np.stack(extra_mats).astype(np.float32) if extra_mats
             else np.zeros((0, C, C), np.float32))
    return offsets, mats, off_specs, extra_offsets, extra


def _build_corrections(w2, w3, w4):
    """24 strip-correction terms (matrices already NEGATED for accumulation).

    Strips j<4: column strips (out col px, read x col src, row shift ty);
    j>=4: row strips. Each strip has 3 taps."""
    w4m = w4[:, :, 0, 0].astype(np.float64)
    strips, mats = [], []
    specs = [
        ("col", 2, 8), ("col", 2, 12), ("col", 3, 12), ("col", 3, 8),
        ("row", 0, 8), ("row", 0, 12), ("row", 1, 12), ("row", 1, 8),
    ]
    for kind, g, d in specs:
        sy, sx = SHIFTS[g]
        sl = slice(g * C, (g + 1) * C)
        w = w2 if d == 8 else w3
        if kind == "col":
            border = -1 if sx == 1 else W
            fixed_out = border - (-d if sx == 1 else d)
            src = border + sx
            shifts = [-d, 0, d]                     # ty values
            tap_b = 0 if sx == 1 else 2
            kvs = [w[sl, 0, a, tap_b] for a in range(3)]
        else:
            border = -1 if sy == 1 else H
            fixed_out = border - (-d if sy == 1 else d)
            src = border + sy
            shifts = [-d, 0, d]                     # tx values
            tap_a = 0 if sy == 1 else 2
            kvs = [w[sl, 0, tap_a, b] for b in range(3)]
        strips.append(dict(kind=kind, fixed_out=fixed_out, src=src, shifts=shifts))
        for kv in kvs:
            mats.append(-(w4m[:, sl] * kv.astype(np.float64)[None, :]))
    return strips, np.stack(mats).astype(np.float32)


def _build_weights(inputs):
    w1, w2, w3, w4 = inputs["w1"], inputs["w2"], inputs["w3"], inputs["w4"]
    b1, b2, b3, b4 = inputs["b1"], inputs["b2"], inputs["b3"], inputs["b4"]
    offsets, mats, off_specs, extra_offsets, extra_mats = _build_terms(
        w1, w2, w3, w4)
    strips, cmats = _build_corrections(w2, w3, w4)
    # fp8 stationary: per-offset fold matrices, scaled into e4m3 range.
    # Saturate at the ml_dtypes float8_e4m3 max (240): values beyond it
    # would cast to inf. No-op for the reference weights (absmax 188).
    # The tail-granule extra matrices are appended after the main terms.
    all_mats = np.concatenate([mats, extra_mats], axis=0)
    wt8 = np.ascontiguousarray(
        np.clip((all_mats * WSCALE), -240.0, 240.0)
        .transpose(2, 0, 1).reshape(C, -1))                      # [C, T*C]
    # bf16 stationary: 24 corrections (scaled) + per-group PW (scaled) +
    # ident (strip folds, x1) + ident*WSCALE (residual-into-psum)
    pw = np.stack([sp["w4g"] for sp in off_specs])               # [n_off, C, C]
    ident = np.eye(C, dtype=np.float32)[None]
    wtb = np.concatenate([cmats * WSCALE, pw * WSCALE, ident,
                          ident * WSCALE], axis=0)
    wtb = np.ascontiguousarray(wtb.transpose(2, 0, 1).reshape(C, -1))
    ks = np.concatenate([sp["kmat"] for sp in off_specs], axis=1)  # [C, ntaps]
    w4m = w4[:, :, 0, 0].astype(np.float64)
    beff = (b4.astype(np.float64)
            + w4m @ (b1 + b2 + b3).astype(np.float64)).astype(np.float32)
    return wt8, wtb, ks, beff, offsets, off_specs, strips, extra_offsets


# --------------------------------------------------------------------------
# device program
# --------------------------------------------------------------------------

_CACHE = {}


def _build_program(offsets, off_specs, strips, extra_offsets):
    import concourse.bacc as bacc
    import concourse.mybir as mybir
    import concourse.tile as tile
    from concourse.ap import AP

    nc = bacc.Bacc("TRN2", target_bir_lowering=False)
    f32 = mybir.dt.float32
    bf16 = mybir.dt.bfloat16
    f8 = mybir.dt.float8e4

    n_terms = len(offsets)
    n_pairs = n_terms // 2
    n_single = n_terms % 2
    n_extra = len(extra_offsets)
    assert n_extra % 2 == 0, "tail-PE taps must pair up"
    n_t8 = n_terms + n_extra
    n_off = len(off_specs)
    n_ks = sum(len(sp["tap_offsets"]) for sp in off_specs)
    # bf16 block indices
    CORR_BLK = 0
    PW_BLK = 24
    ID_BLK = 24 + n_off
    RES_BLK = 24 + n_off + 1
    nb_blk = 24 + n_off + 2

    xp8_d = nc.dram_tensor("xp8", [C, HP * WP], f8, kind="ExternalInput")
    xpb_d = nc.dram_tensor("xpb", [C, HP * WP], bf16, kind="ExternalInput")
    # the 4 border lines the strip corrections read (2 padded cols + 2
    # padded rows), packed so the corrections don't wait on the full image
    xbrd_d = nc.dram_tensor("xbrd", [C, 2 * HP + 2 * WP], bf16,
                            kind="ExternalInput")
    wt8_d = nc.dram_tensor("wt8", [C, n_t8 * C], f8, kind="ExternalInput")
    wtb_d = nc.dram_tensor("wtb", [C, nb_blk * C], bf16, kind="ExternalInput")
    ks_d = nc.dram_tensor("ks", [C, n_ks], f32, kind="ExternalInput")
    beff_d = nc.dram_tensor("beff", [C, 1], f32, kind="ExternalInput")
    out_d = nc.dram_tensor("out", [C, H * W], f32, kind="ExternalOutput")

    with tile.TileContext(nc) as tc:
        with (
            tc.tile_pool(name="const", bufs=1) as const,
            tc.tile_pool(name="outp", bufs=2) as outp,
            tc.tile_pool(name="yp", bufs=3) as yp,
            tc.tile_pool(name="up", bufs=2) as up,
            tc.tile_pool(name="psum", bufs=4, space="PSUM") as psum_pool,
        ):
            xp8_sb = const.tile([C, HP * WP], f8)
            xpb_sb = const.tile([C, HP * WP], bf16)
            xbrd_sb = const.tile([C, 2 * HP + 2 * WP], bf16)
            wt8_sb = const.tile([C, n_t8 * C], f8)
            wtb_sb = const.tile([C, nb_blk * C], bf16)
            ks_sb = const.tile([C, n_ks], f32)
            beff_sb = const.tile([C, 1], f32)

            # SWDGE (nc.gpsimd) fans >=1MB transfers across all 16 SDMA
            # engines (~340 GB/s); HWDGE runs ~26 GB/s on a single engine.
            # Order: minimum needed for SB0 first, then all of xpb (the
            # SB0-time corrections read the whole bf16 image), then xp8.
            # DMA order = the two critical chains' first needs, interleaved:
            # PE's first matmuls need the first weight pairs + SB0's fp8
            # rows; the DVE/ACT tap queue needs xpb's first granule + ks.
            nc.sync.dma_start(out=ks_sb, in_=ks_d[:, :])
            W8SPLIT = min(16 * C, n_t8 * C)
            nc.gpsimd.dma_start(out=wt8_sb[:, :W8SPLIT],
                                in_=wt8_d[:, :W8SPLIT])
            # SB0's first pairs read xp8 rows 1..~20 only; the rest of the
            # 36-row context follows in a second chunk
            ROWS0 = SB_ROWS + 2 * PAD
            ROWS0A = 20
            nc.gpsimd.dma_start(out=xp8_sb[:, :ROWS0A * WP],
                                in_=xp8_d[:, :ROWS0A * WP])
            nc.gpsimd.dma_start(out=xp8_sb[:, ROWS0A * WP:ROWS0 * WP],
                                in_=xp8_d[:, ROWS0A * WP:ROWS0 * WP])
            nc.gpsimd.dma_start(out=xpb_sb[:, :(2 * SB_ROWS + 2 * PAD) * WP],
                                in_=xpb_d[:, :(2 * SB_ROWS + 2 * PAD) * WP])
            # xbrd is only read by the corrections at the end of SB0
            # (~30us in) -- keep it off the startup critical path
            nc.gpsimd.dma_start(out=xbrd_sb, in_=xbrd_d[:, :])
            nc.gpsimd.dma_start(out=wt8_sb[:, W8SPLIT:],
                                in_=wt8_d[:, W8SPLIT:])
            nc.gpsimd.dma_start(out=wtb_sb, in_=wtb_d[:, :])
            XP_CHUNK_ROWS = 48
            for r0_ in range(2 * SB_ROWS + 2 * PAD, HP, XP_CHUNK_ROWS):
                r1_ = min(r0_ + XP_CHUNK_ROWS, HP)
                nc.gpsimd.dma_start(out=xpb_sb[:, r0_ * WP:r1_ * WP],
                                    in_=xpb_d[:, r0_ * WP:r1_ * WP])
            for r0_ in range(ROWS0, HP, XP_CHUNK_ROWS):
                r1_ = min(r0_ + XP_CHUNK_ROWS, HP)
                nc.gpsimd.dma_start(out=xp8_sb[:, r0_ * WP:r1_ * WP],
                                    in_=xp8_d[:, r0_ * WP:r1_ * WP])
            nc.sync.dma_start(out=beff_sb, in_=beff_d[:, :])

            xp3 = xp8_sb.rearrange("p (r w) -> p r w", w=WP)

            def wblk8_pair(p):
                return wt8_sb[:, 2 * p * C:(2 * p + 2) * C].rearrange(
                    "p (two m) -> p two m", two=2)

            def wblk8(i):
                return wt8_sb[:, i * C:(i + 1) * C]

            def wblkb(i):
                return wtb_sb[:, i * C:(i + 1) * C]

            def pair_rhs(o_a, o_b, a0, sub):
                """rhs AP [C, 2, sub, W]: two shifted views, pair delta."""
                va = xp3[:, a0 + o_a[0]: a0 + o_a[0] + sub,
                         PAD + o_a[1]: PAD + o_a[1] + W]
                delta = (o_b[0] - o_a[0]) * WP + (o_b[1] - o_a[1])
                ap = list(va.ap)
                ap = [ap[0], (delta, 2), ap[1], ap[2]]
                return AP(tensor=va.tensor, offset=va.offset, ap=ap)

            corr_sb = const.tile([C, 8 * H], bf16)

            xb3 = xpb_sb.rearrange("p (r w) -> p r w", w=WP)

            def emit_corrections():
                # reads the packed border tensor (bf16 weights x bf16
                # moving; do not mix dtypes in one mm).  Borrows a main-psum
                # rotation slot (same tag+size) so all 8 PSUM banks serve
                # the pipeline.
                psum_c = psum_pool.tile([C, 8 * H], f32, name="psum_c",
                                        tag="acc")
                for j, st in enumerate(strips):
                    if st["kind"] == "col":
                        base = 0 if st["src"] == 0 else HP
                        n = H
                    else:
                        base = 2 * HP + (0 if st["src"] == 0 else WP)
                        n = W
                    for i, sh in enumerate(st["shifts"]):
                        rhs = xbrd_sb[:, base + PAD + sh:
                                      base + PAD + sh + n]
                        nc.tensor.matmul(psum_c[:, j * H:(j + 1) * H],
                                         wblkb(CORR_BLK + 3 * j + i), rhs,
                                         start=(i == 0), stop=(i == 2))
                # ACT, not DVE: DVE is busy with taps; psum_c slot release
                # should not sit behind them
                nc.scalar.copy(corr_sb, psum_c)

            # per-OFFLOAD-group scalar column base in ks
            ks_base = []
            b = 0
            for sp in off_specs:
                ks_base.append(b)
                b += len(sp["tap_offsets"])

            # ---- main loop -------------------------------------------------
            n_sub = SB_ROWS // SUB_ROWS
            SB_PER_G = 2          # tap FMAs at 2-SB granularity; 4-SB lumps
            Y_ROWS = SB_PER_G * SB_ROWS   # regressed (coarser pipeline)

            def emit_taps(gi):
                """Tap FMAs for granule gi (SBs 2*gi, 2*gi+1). Multiplies:
                DVE 4x-mode on contiguous padded-width spans (row-wrap
                garbage stays in pad cols since |dx|<PAD) or ACT activation
                Copy with per-partition scale (own SBUF port, no DVE
                contention). Accumulation on the center views: DVE 2x
                tensor_tensor. ACT products alternate between two ua tiles
                so the ACT muls and DVE adds pipeline instead of ping-pong
                serializing on one buffer. Returns [(y3_view, pw_block)]."""
                r0 = SB_PER_G * gi * SB_ROWS
                last_gran = gi == N_SB // SB_PER_G - 1
                pair_ys = []
                for oi, sp in enumerate(off_specs):
                    g = sp["g"]
                    taps = sp["tap_offsets"]
                    act_set = sp["act_set"]
                    nd = len(taps)
                    live = [t for t in range(nd)
                            if not (last_gran and t in sp["tail_pe"])]
                    if not live:
                        continue  # all taps on the PE for the tail SBs

                    def span(t_idx):
                        dy, dx = taps[t_idx]
                        off0 = (PAD + r0 + dy) * WP + dx
                        return xpb_sb[:, off0: off0 + Y_ROWS * WP]

                    def center(t_idx):
                        dy, dx = taps[t_idx]
                        return xb3[:, PAD + r0 + dy: PAD + r0 + dy + Y_ROWS,
                                   PAD + dx: PAD + dx + W]

                    def kcol(t_idx):
                        cb = ks_base[oi] + t_idx
                        return ks_sb[:, cb:cb + 1]

                    # deep y rotation lets the tap engines start a granule as
                    # soon as possible -- the y-buffer WAR release (not DVE
                    # total work) gates the pipeline tail
                    y = yp.tile([C, Y_ROWS * WP], bf16, tag=f"y{g}",
                                bufs=6 if g == 4 else 4)
                    y3 = y.rearrange("p (r w) -> p r w", w=WP)
                    yc = y3[:, :, PAD:PAD + W]
                    u = u3 = None
                    if any(t not in act_set for t in live):
                        u = up.tile([C, Y_ROWS * WP], bf16, tag=f"u{g}",
                                    bufs=3)
                        u3 = u.rearrange("p (r w) -> p r w", w=WP)
                    uas, na = [], 0
                    if act_set:
                        # ACT products cover only the consumed 128-col
                        # center (ACT has no perf modes -- cost is element
                        # count; the DVE muls keep full contiguous spans
                        # for 4x mode)
                        for j in range(2):
                            ua = up.tile([C, Y_ROWS * W], bf16,
                                         tag=f"ua{g}{j}", bufs=2)
                            uas.append(
                                (ua, ua.rearrange("p (r w) -> p r w", w=W)))

                    t0 = live[0]
                    if t0 in act_set:
                        # center-only init: y's pad columns are never read
                        nc.scalar.activation(
                            yc, center(t0),
                            mybir.ActivationFunctionType.Copy,
                            scale=kcol(t0))
                    else:
                        nc.vector.tensor_scalar_mul(y, span(t0), kcol(t0))
                    for t in live[1:]:
                        if t in act_set:
                            flat, v3 = uas[na % 2]
                            na += 1
                            nc.scalar.activation(
                                flat, center(t),
                                mybir.ActivationFunctionType.Copy,
                                scale=kcol(t))
                            src = v3
                        else:
                            nc.vector.tensor_scalar_mul(u, span(t), kcol(t))
                            src = u3[:, :, PAD:PAD + W]
                        nc.vector.tensor_tensor(
                            yc, yc, src, mybir.AluOpType.add)
                    pair_ys.append((y3, PW_BLK + oi))
                return pair_ys

            granule_ys = {0: emit_taps(0)}
            for s in range(N_SB):
                r0 = s * SB_ROWS
                half = (s % SB_PER_G) * SB_ROWS
                ys = [(y3[:, half:half + SB_ROWS, PAD:PAD + W], blk)
                      for y3, blk in granule_ys[s // SB_PER_G]]

                psum = psum_pool.tile([C, SB_ROWS * W], f32, tag="acc")
                pair_list = [(offsets[2 * p], offsets[2 * p + 1], p)
                             for p in range(n_pairs)]
                if s // SB_PER_G == N_SB // SB_PER_G - 1:
                    pair_list += [
                        (extra_offsets[2 * j], extra_offsets[2 * j + 1],
                         n_pairs + j) for j in range(n_extra // 2)]
                for i, (o_a, o_b, p) in enumerate(pair_list):
                    for u_ in range(n_sub):
                        a0 = PAD + r0 + u_ * SUB_ROWS
                        nc.tensor.matmul(
                            psum[:, u_ * SUB_ROWS * W:(u_ + 1) * SUB_ROWS * W],
                            wblk8_pair(p), pair_rhs(o_a, o_b, a0, SUB_ROWS),
                            start=(i == 0), stop=False,
                            perf_mode=mybir.MatmulPerfMode.DoubleRow)
                if n_single:
                    di, dj = offsets[-1]
                    for u_ in range(n_sub):
                        a0 = PAD + r0 + u_ * SUB_ROWS + di
                        nc.tensor.matmul(
                            psum[:, u_ * SUB_ROWS * W:(u_ + 1) * SUB_ROWS * W],
                            wblk8(n_terms - 1),
                            xp3[:, a0: a0 + SUB_ROWS, PAD + dj: PAD + dj + W],
                            start=False, stop=False)
                for yv, blk in ys:
                    for u_ in range(n_sub):
                        nc.tensor.matmul(
                            psum[:, u_ * SUB_ROWS * W:(u_ + 1) * SUB_ROWS * W],
                            wblkb(blk),
                            yv[:, u_ * SUB_ROWS:(u_ + 1) * SUB_ROWS, :],
                            start=False, stop=False)

                if s == 0:
                    emit_corrections()
                # prefetch next granule's taps ahead of this SB's
                # evacuation so the DVE/ACT/GpSimd queues never sit behind
                # the psum drain
                if s % SB_PER_G == 0 and s // SB_PER_G + 1 < N_SB // SB_PER_G:
                    granule_ys[s // SB_PER_G + 1] = emit_taps(s // SB_PER_G + 1)

                # fold strip corrections into PSUM on the PE: identity-weight
                # matmuls add corr_sb rows into strided psum positions
                psum3 = psum.rearrange("p (r w) -> p r w", w=W)
                # all 4 col strips in ONE ident matmul: their psum columns
                # {7,11,116,120} factor as c0 + a*da + b*db with the strips
                # laid out j = 2a+b in corr_sb -> congruent 4-dim APs
                cols = [st["fixed_out"] for st in strips if st["kind"] == "col"]
                c0, db, da = cols[0], cols[1] - cols[0], cols[2] - cols[0]
                assert cols == [c0, c0 + db, c0 + da, c0 + da + db]
                src0 = corr_sb[:, r0: r0 + SB_ROWS]
                rhs4 = AP(tensor=src0.tensor, offset=src0.offset,
                          ap=[src0.ap[0], (2 * H, 2), (H, 2), (1, SB_ROWS)])
                dst0 = psum[:, c0:c0 + 1]
                out4 = AP(tensor=dst0.tensor, offset=dst0.offset,
                          ap=[dst0.ap[0], (da, 2), (db, 2), (W, SB_ROWS)])
                strip_mms = [(out4, rhs4)]
                for j, st in enumerate(strips):
                    if st["kind"] == "row" and r0 <= st["fixed_out"] < r0 + SB_ROWS:
                        lr = st["fixed_out"] - r0
                        strip_mms.append((psum3[:, lr:lr + 1, :],
                                          corr_sb[:, j * H: j * H + W]))
                for i, (dst, src) in enumerate(strip_mms):
                    nc.tensor.matmul(dst, wblkb(ID_BLK), src,
                                     start=False, stop=False)
                # residual into PSUM: ident*WSCALE applied to the bf16 image
                # (0.4% of |x|, inside the error budget; saves the fp32 x DMA
                # and keeps the whole evacuation off the busy Vector engine)
                for u_ in range(n_sub):
                    a0 = PAD + r0 + u_ * SUB_ROWS
                    nc.tensor.matmul(
                        psum[:, u_ * SUB_ROWS * W:(u_ + 1) * SUB_ROWS * W],
                        wblkb(RES_BLK),
                        xb3[:, a0: a0 + SUB_ROWS, PAD:PAD + W],
                        start=False, stop=(u_ == n_sub - 1))

                # single DVE dual-op: out = (psum + beff*4096) * 2^-12.
                # (ACT would need two ops -- HW drops the scale when scale
                # and bias are combined -- and the DVE has the spare
                # capacity now that ACT carries most tap multiplies.)
                out_sb = outp.tile([C, SB_ROWS * W], f32)
                nc.vector.tensor_scalar(out_sb, psum, beff_sb[:, 0:1],
                                        1.0 / WSCALE,
                                        mybir.AluOpType.add,
                                        mybir.AluOpType.mult)
                nc.gpsimd.dma_start(out=out_d[:, r0 * W:(r0 + SB_ROWS) * W],
                                    in_=out_sb)
    nc.finalize()
    return nc


def _make_in_maps(inputs):
    x = np.ascontiguousarray(inputs["x"], dtype=np.float32)
    (wt8, wtb, ks, beff, offsets, off_specs, strips,
     extra_offsets) = _build_weights(inputs)
    if "nc" not in _CACHE:
        _CACHE["nc"] = _build_program(offsets, off_specs, strips,
                                      extra_offsets)

    import ml_dtypes
    bf = ml_dtypes.bfloat16
    f8 = ml_dtypes.float8_e4m3
    xpad8 = np.zeros((B, C, HP, WP), f8)
    xpad8[:, :, PAD:PAD + H, PAD:PAD + W] = x.astype(f8)
    xpadb = np.zeros((B, C, HP, WP), bf)
    xpadb[:, :, PAD:PAD + H, PAD:PAD + W] = x.astype(bf)
    beff_col = np.ascontiguousarray((beff * WSCALE).reshape(C, 1))
    wt8_f8 = wt8.astype(f8)
    wtb_bf = wtb.astype(bf)
    ksc = np.ascontiguousarray(ks)
    return [
        {
            "xp8": np.ascontiguousarray(xpad8[b].reshape(C, HP * WP)),
            "xpb": np.ascontiguousarray(xpadb[b].reshape(C, HP * WP)),
            "xbrd": np.ascontiguousarray(np.concatenate(
                [xpadb[b, :, :, PAD + 0], xpadb[b, :, :, PAD + W - 1],
                 xpadb[b, :, PAD + 0, :], xpadb[b, :, PAD + H - 1, :]],
                axis=1)),
            "wt8": wt8_f8,
            "wtb": wtb_bf,
            "ks": ksc,
            "beff": beff_col,
        }
        for b in range(B)
    ]


def kernel(**inputs):
    in_maps = _make_in_maps(inputs)
    from concourse.bass_utils import run_bass_kernel_spmd
    res = run_bass_kernel_spmd(_CACHE["nc"], in_maps, core_ids=list(range(N_CORES)))
    out = np.stack([res.results[b]["out"].reshape(C, H, W) for b in range(B)])
    return out.astype(np.float32)



# revision 49
# speedup vs baseline: 1.3496x; 1.0200x over previous
"""Trainium2 Bass kernel for nn_LongRangeDW (dense_cnn).

The module is entirely linear in x:
  s = nnstacking(x)                        (5 shifted copies, clipped to window)
  y = dw1(s) + dw2(s) + dw3(s)             (depthwise 1x1 + 3x3 d8 + 3x3 d12)
  out = pw(y) + x                          (pointwise 5C->C + residual)

Folding the depthwise taps into the pointwise gives, per nnstacking group g
with shift sigma_g and tap tau:
  out[o, p] = sum_{g,t} (W4_g diag(k_{g,t}))[o,:] @ xe[:, p + tau_t + sigma_g]
              + beff[o] + x[o, p]
with xe = zero-extended x: 85 distinct offsets. The non-offloaded offsets run
as fp8 DoubleRow matmul PAIRS on the tensor engine: two 128x128 fp8 matrices
(scaled by 2^12 into e4m3 range) stream two shifted image views together at
2 column-pairs/cycle -- half the bf16 cost per term. The pair's second view
is expressed directly as an AP [K, 2, rows, W] whose dim-1 stride is the
offset delta into the padded fp8 image.

21 taps (group 4's 17 + 4 of group 0's) are offloaded to the Vector+Scalar
engines as per-channel-scalar FMAs on a bf16 copy of the image: 9 multiplies
run in the DVE 4x perf mode on fully contiguous padded-width spans (|dx| <=
PAD keeps row-wrap garbage inside the pad columns), 12 on the Scalar engine
(activation Copy with per-partition scale, center span only -- ACT has its
own SBUF port so it never contends with the DVE, unlike GpSimd which shares
the DVE's second port under an exclusive full-instruction lock).
Accumulation is DVE 2x tensor_tensor on the center views.  Each offloaded
group's y feeds one bf16 pointwise matmul per sub-block.  Offloading work
off the PE also eased the chip power throttle (util limit 0.91 -> ~1.0),
speeding every remaining matmul ~17%.

The tap pipeline (not the PE) gates the kernel tail, so: y tiles rotate 6
deep (the y-buffer WAR release paces when a granule can start), the bf16
image + per-tap scalars are the first DMAs issued, and for the LAST granule
most taps run on the PE instead (extra fp8 pairs) so the final granule is
short.  Evacuation is a single DVE dual-op tensor_scalar
(out = (psum + 4096*beff) * 2^-12); psum rotates over all 8 banks (4 tiles).
The boundary-correction matmuls read a tiny host-packed border tensor so
they need not wait for the full image DMA.

Boundary exactness: composing clipped shifts with zero-padded convs is NOT the
padded composite. Where a depthwise tap lands exactly 1 px outside the window
and sigma_g pulls it back in, the composite wrongly reads x. The mismatch
lives on 8 one-pixel strips (output rows/cols {7,11,116,120}) reading x's 4
border lines -> 24 small correction matmuls folded in during evacuation.

The residual enters PSUM as an ident*2^12 matmul of the bf16 image (0.4% of
|x|, inside the error budget; saves the 8.4MB fp32 x transfer).

Measured on trn2: 293.4us (from the 344us two-engine version and 477us for
the all-bf16 single-engine version).

Data parallel: batch B=8 -> one image per NeuronCore.
"""

import sys

import numpy as np

sys.path.insert(0, "/opt/trn_rl_repo")

B, C, H, W = 8, 128, 128, 128
PAD = 14            # max |offset| = 13, rounded even for DVE 4B alignment
HP = H + 2 * PAD
WP = W + 2 * PAD
N_CORES = 8
SB_ROWS = 8         # output rows per super-block (psum tile = 2 banks)
N_SB = H // SB_ROWS
SUB_ROWS = 4        # rows per matmul (out free dim 512 = one PSUM bank)

WSCALE = 4096.0     # fp8 weight scale (2^12); removed at evacuation

SHIFTS = [(1, 0), (-1, 0), (0, 1), (0, -1), (0, 0)]  # nnstacking groups

# (group, n_taps, act_mul_tap_indices): taps offloaded off the PE.  All
# multiplies whose tap index is in the act set run on the Scalar engine
# (activation Copy with per-partition scale, dedicated SBUF port); the rest
# run on the Vector engine in 4x mode (needs even dx -> groups 0, 1, 4).
# Accumulation is always Vector tensor_tensor (2x mode).  GpSimd compute is
# OFF-LIMITS: it arbitrates for the same shared SBUF port pair the DVE perf
# modes need (exclusive full-instruction lock) -- the v4 experiment that put
# adds there dropped DVE to 2x and slowed the kernel to 477us.
OFFLOAD = [(4, 17, frozenset({1, 3, 5, 7, 9, 11, 13, 15})),
           (0, 4, frozenset({0, 1, 2, 3}))]
# Per offloaded group: tap indices that run on the PE (as extra fp8 pairs)
# for the LAST tap granule only: shrinks the final granule's DVE/ACT
# latency, which otherwise leaves the PE idling ~11us at the end of the
# pipeline (no pair-matmul work left to overlap the last granule's taps).
# The total tap count across groups must be even (fp8 pairs).
TAIL_PE_TAPS = {0: tuple(range(4)),
                4: (0, 2, 4, 6, 8, 10, 12, 14)}


# --------------------------------------------------------------------------
# host-side operator folding
# --------------------------------------------------------------------------

def _group_taps(w1, w2, w3, g):
    """All 17 taps of group g as {(di, dj): kvec[C]} (shift folded in)."""
    sy, sx = SHIFTS[g]
    sl = slice(g * C, (g + 1) * C)
    taps = {}

    def add(di, dj, kv):
        v = taps.setdefault((di, dj), np.zeros(C, np.float64))
        v += kv.astype(np.float64)

    add(sy, sx, w1[sl, 0, 0, 0])
    for w, d in ((w2, 8), (w3, 12)):
        for a in range(3):
            for b in range(3):
                add(sy + (a - 1) * d, sx + (b - 1) * d, w[sl, 0, a, b])
    return taps


def _build_terms(w1, w2, w3, w4):
    """Returns (offsets, mats, off_specs, extra_offsets, extra_mats).
    off_specs is a list per OFFLOAD entry: dict(g, tap_offsets, kmat [C, n],
    act_set, w4g [C, C]).  extra_* are per-tap fold matrices for the
    TAIL_PE_GROUPS offloaded taps (used only for the last granule's SBs)."""
    w4m = w4[:, :, 0, 0].astype(np.float64)  # [C, 5C]
    offload_n = {g: (n, act) for g, n, act in OFFLOAD}
    mat_terms = {}
    off_specs = []
    extra_offsets, extra_mats = [], []
    for g in range(5):
        taps = _group_taps(w1, w2, w3, g)
        tap_offsets = sorted(taps)
        n_off, act_set = offload_n.get(g, (0, frozenset()))
        off, keep = tap_offsets[:n_off], tap_offsets[n_off:]
        if off:
            kmat = np.stack([taps[o] for o in off], axis=1)  # [C, n]
            tail_pe = tuple(t for t in TAIL_PE_TAPS.get(g, ()) if t < len(off))
            off_specs.append(dict(
                g=g, tap_offsets=off, kmat=kmat.astype(np.float32),
                act_set=act_set, tail_pe=frozenset(tail_pe),
                w4g=w4m[:, g * C:(g + 1) * C].astype(np.float32)))
            for t in tail_pe:
                o = off[t]
                extra_offsets.append(o)
                extra_mats.append(
                    w4m[:, g * C:(g + 1) * C] * taps[o][None, :])
        for o in keep:
            M = mat_terms.setdefault(o, np.zeros((C, C), np.float64))
            M += w4m[:, g * C:(g + 1) * C] * taps[o][None, :]
    offsets = sorted(mat_terms)
    mats = np.stack([mat_terms[o] for o in offsets]).astype(np.float32)
    extra = (np.stack(extra_mats).astype(np.float32) if extra_mats
             else np.zeros((0, C, C), np.float32))
    return offsets, mats, off_specs, extra_offsets, extra


def _build_corrections(w2, w3, w4):
    """24 strip-correction terms (matrices already NEGATED for accumulation).

    Strips j<4: column strips (out col px, read x col src, row shift ty);
    j>=4: row strips. Each strip has 3 taps."""
    w4m = w4[:, :, 0, 0].astype(np.float64)
    strips, mats = [], []
    specs = [
        ("col", 2, 8), ("col", 2, 12), ("col", 3, 12), ("col", 3, 8),
        ("row", 0, 8), ("row", 0, 12), ("row", 1, 12), ("row", 1, 8),
    ]
    for kind, g, d in specs:
        sy, sx = SHIFTS[g]
        sl = slice(g * C, (g + 1) * C)
        w = w2 if d == 8 else w3
        if kind == "col":
            border = -1 if sx == 1 else W
            fixed_out = border - (-d if sx == 1 else d)
            src = border + sx
            shifts = [-d, 0, d]                     # ty values
            tap_b = 0 if sx == 1 else 2
            kvs = [w[sl, 0, a, tap_b] for a in range(3)]
        else:
            border = -1 if sy == 1 else H
            fixed_out = border - (-d if sy == 1 else d)
            src = border + sy
            shifts = [-d, 0, d]                     # tx values
            tap_a = 0 if sy == 1 else 2
            kvs = [w[sl, 0, tap_a, b] for b in range(3)]
        strips.append(dict(kind=kind, fixed_out=fixed_out, src=src, shifts=shifts))
        for kv in kvs:
            mats.append(-(w4m[:, sl] * kv.astype(np.float64)[None, :]))
    return strips, np.stack(mats).astype(np.float32)


def _build_weights(inputs):
    w1, w2, w3, w4 = inputs["w1"], inputs["w2"], inputs["w3"], inputs["w4"]
    b1, b2, b3, b4 = inputs["b1"], inputs["b2"], inputs["b3"], inputs["b4"]
    offsets, mats, off_specs, extra_offsets, extra_mats = _build_terms(
        w1, w2, w3, w4)
    strips, cmats = _build_corrections(w2, w3, w4)
    # fp8 stationary: per-offset fold matrices, scaled into e4m3 range.
    # Saturate at the ml_dtypes float8_e4m3 max (240): values beyond it
    # would cast to inf. No-op for the reference weights (absmax 188).
    # The tail-granule extra matrices are appended after the main terms.
    all_mats = np.concatenate([mats, extra_mats], axis=0)
    wt8 = np.ascontiguousarray(
        np.clip((all_mats * WSCALE), -240.0, 240.0)
        .transpose(2, 0, 1).reshape(C, -1))                      # [C, T*C]
    # bf16 stationary: 24 corrections (scaled) + per-group PW (scaled) +
    # ident (strip folds, x1) + ident*WSCALE (residual-into-psum)
    pw = np.stack([sp["w4g"] for sp in off_specs])               # [n_off, C, C]
    ident = np.eye(C, dtype=np.float32)[None]
    wtb = np.concatenate([cmats * WSCALE, pw * WSCALE, ident,
                          ident * WSCALE], axis=0)
    wtb = np.ascontiguousarray(wtb.transpose(2, 0, 1).reshape(C, -1))
    ks = np.concatenate([sp["kmat"] for sp in off_specs], axis=1)  # [C, ntaps]
    w4m = w4[:, :, 0, 0].astype(np.float64)
    beff = (b4.astype(np.float64)
            + w4m @ (b1 + b2 + b3).astype(np.float64)).astype(np.float32)
    return wt8, wtb, ks, beff, offsets, off_specs, strips, extra_offsets


# --------------------------------------------------------------------------
# device program
# --------------------------------------------------------------------------

_CACHE = {}


def _build_program(offsets, off_specs, strips, extra_offsets):
    import concourse.bacc as bacc
    import concourse.mybir as mybir
    import concourse.tile as tile
    from concourse.ap import AP

    nc = bacc.Bacc("TRN2", target_bir_lowering=False)
    f32 = mybir.dt.float32
    bf16 = mybir.dt.bfloat16
    f8 = mybir.dt.float8e4

    n_terms = len(offsets)
    n_pairs = n_terms // 2
    n_single = n_terms % 2
    n_extra = len(extra_offsets)
    assert n_extra % 2 == 0, "tail-PE taps must pair up"
    n_t8 = n_terms + n_extra
    n_off = len(off_specs)
    n_ks = sum(len(sp["tap_offsets"]) for sp in off_specs)
    # bf16 block indices
    CORR_BLK = 0
    PW_BLK = 24
    ID_BLK = 24 + n_off
    RES_BLK = 24 + n_off + 1
    nb_blk = 24 + n_off + 2

    xp8_d = nc.dram_tensor("xp8", [C, HP * WP], f8, kind="ExternalInput")
    xpb_d = nc.dram_tensor("xpb", [C, HP * WP], bf16, kind="ExternalInput")
    # the 4 border lines the strip corrections read (2 padded cols + 2
    # padded rows), packed so the corrections don't wait on the full image
    xbrd_d = nc.dram_tensor("xbrd", [C, 2 * HP + 2 * WP], bf16,
                            kind="ExternalInput")
    wt8_d = nc.dram_tensor("wt8", [C, n_t8 * C], f8, kind="ExternalInput")
    wtb_d = nc.dram_tensor("wtb", [C, nb_blk * C], bf16, kind="ExternalInput")
    ks_d = nc.dram_tensor("ks", [C, n_ks], f32, kind="ExternalInput")
    beff_d = nc.dram_tensor("beff", [C, 1], f32, kind="ExternalInput")
    out_d = nc.dram_tensor("out", [C, H * W], f32, kind="ExternalOutput")

    with tile.TileContext(nc) as tc:
        with (
            tc.tile_pool(name="const", bufs=1) as const,
            tc.tile_pool(name="outp", bufs=2) as outp,
            tc.tile_pool(name="yp", bufs=3) as yp,
            tc.tile_pool(name="up", bufs=2) as up,
            tc.tile_pool(name="psum", bufs=4, space="PSUM") as psum_pool,
        ):
            xp8_sb = const.tile([C, HP * WP], f8)
            xpb_sb = const.tile([C, HP * WP], bf16)
            xbrd_sb = const.tile([C, 2 * HP + 2 * WP], bf16)
            wt8_sb = const.tile([C, n_t8 * C], f8)
            wtb_sb = const.tile([C, nb_blk * C], bf16)
            ks_sb = const.tile([C, n_ks], f32)
            beff_sb = const.tile([C, 1], f32)

            # SWDGE (nc.gpsimd) fans >=1MB transfers across all 16 SDMA
            # engines (~340 GB/s); HWDGE runs ~26 GB/s on a single engine.
            # Order: minimum needed for SB0 first, then all of xpb (the
            # SB0-time corrections read the whole bf16 image), then xp8.
            # DMA order = the two critical chains' first needs, interleaved:
            # PE's first matmuls need the first weight pairs + SB0's fp8
            # rows; the DVE/ACT tap queue needs xpb's first granule + ks.
            nc.sync.dma_start(out=ks_sb, in_=ks_d[:, :])
            W8SPLIT = min(16 * C, n_t8 * C)
            nc.gpsimd.dma_start(out=wt8_sb[:, :W8SPLIT],
                                in_=wt8_d[:, :W8SPLIT])
            ROWS0 = SB_ROWS + 2 * PAD
            nc.gpsimd.dma_start(out=xp8_sb[:, :ROWS0 * WP],
                                in_=xp8_d[:, :ROWS0 * WP])
            nc.gpsimd.dma_start(out=xpb_sb[:, :(2 * SB_ROWS + 2 * PAD) * WP],
                                in_=xpb_d[:, :(2 * SB_ROWS + 2 * PAD) * WP])
            # xbrd is only read by the corrections at the end of SB0
            # (~30us in) -- keep it off the startup critical path.  NOTE:
            # issuing it as the FIRST SWDGE transfer (before wt8/xp8)
            # produced full-output NaNs on cores 1-7 in 3/4 runs; keep it
            # here, after the startup-critical chunks.
            nc.gpsimd.dma_start(out=xbrd_sb, in_=xbrd_d[:, :])
            nc.gpsimd.dma_start(out=wt8_sb[:, W8SPLIT:],
                                in_=wt8_d[:, W8SPLIT:])
            nc.gpsimd.dma_start(out=wtb_sb, in_=wtb_d[:, :])
            XP_CHUNK_ROWS = 48
            for r0_ in range(2 * SB_ROWS + 2 * PAD, HP, XP_CHUNK_ROWS):
                r1_ = min(r0_ + XP_CHUNK_ROWS, HP)
                nc.gpsimd.dma_start(out=xpb_sb[:, r0_ * WP:r1_ * WP],
                                    in_=xpb_d[:, r0_ * WP:r1_ * WP])
            for r0_ in range(ROWS0, HP, XP_CHUNK_ROWS):
                r1_ = min(r0_ + XP_CHUNK_ROWS, HP)
                nc.gpsimd.dma_start(out=xp8_sb[:, r0_ * WP:r1_ * WP],
                                    in_=xp8_d[:, r0_ * WP:r1_ * WP])
            nc.sync.dma_start(out=beff_sb, in_=beff_d[:, :])

            xp3 = xp8_sb.rearrange("p (r w) -> p r w", w=WP)

            def wblk8_pair(p):
                return wt8_sb[:, 2 * p * C:(2 * p + 2) * C].rearrange(
                    "p (two m) -> p two m", two=2)

            def wblk8(i):
                return wt8_sb[:, i * C:(i + 1) * C]

            def wblkb(i):
                return wtb_sb[:, i * C:(i + 1) * C]

            def pair_rhs(o_a, o_b, a0, sub):
                """rhs AP [C, 2, sub, W]: two shifted views, pair delta."""
                va = xp3[:, a0 + o_a[0]: a0 + o_a[0] + sub,
                         PAD + o_a[1]: PAD + o_a[1] + W]
                delta = (o_b[0] - o_a[0]) * WP + (o_b[1] - o_a[1])
                ap = list(va.ap)
                ap = [ap[0], (delta, 2), ap[1], ap[2]]
                return AP(tensor=va.tensor, offset=va.offset, ap=ap)

            corr_sb = const.tile([C, 8 * H], bf16)

            xb3 = xpb_sb.rearrange("p (r w) -> p r w", w=WP)

            def emit_corrections():
                # reads the packed border tensor (bf16 weights x bf16
                # moving; do not mix dtypes in one mm).  Borrows a main-psum
                # rotation slot (same tag+size) so all 8 PSUM banks serve
                # the pipeline.
                psum_c = psum_pool.tile([C, 8 * H], f32, name="psum_c",
                                        tag="acc")
                for j, st in enumerate(strips):
                    if st["kind"] == "col":
                        base = 0 if st["src"] == 0 else HP
                        n = H
                    else:
                        base = 2 * HP + (0 if st["src"] == 0 else WP)
                        n = W
                    for i, sh in enumerate(st["shifts"]):
                        rhs = xbrd_sb[:, base + PAD + sh:
                                      base + PAD + sh + n]
                        nc.tensor.matmul(psum_c[:, j * H:(j + 1) * H],
                                         wblkb(CORR_BLK + 3 * j + i), rhs,
                                         start=(i == 0), stop=(i == 2))
                # ACT, not DVE: DVE is busy with taps; psum_c slot release
                # should not sit behind them
                nc.scalar.copy(corr_sb, psum_c)

            # per-OFFLOAD-group scalar column base in ks
            ks_base = []
            b = 0
            for sp in off_specs:
                ks_base.append(b)
                b += len(sp["tap_offsets"])

            # ---- main loop -------------------------------------------------
            n_sub = SB_ROWS // SUB_ROWS
            SB_PER_G = 2          # tap FMAs at 2-SB granularity; 4-SB lumps
            Y_ROWS = SB_PER_G * SB_ROWS   # regressed (coarser pipeline)

            def emit_taps(gi):
                """Tap FMAs for granule gi (SBs 2*gi, 2*gi+1). Multiplies:
                DVE 4x-mode on contiguous padded-width spans (row-wrap
                garbage stays in pad cols since |dx|<PAD) or ACT activation
                Copy with per-partition scale (own SBUF port, no DVE
                contention). Accumulation on the center views: DVE 2x
                tensor_tensor. ACT products alternate between two ua tiles
                so the ACT muls and DVE adds pipeline instead of ping-pong
                serializing on one buffer. Returns [(y3_view, pw_block)]."""
                r0 = SB_PER_G * gi * SB_ROWS
                last_gran = gi == N_SB // SB_PER_G - 1
                pair_ys = []
                for oi, sp in enumerate(off_specs):
                    g = sp["g"]
                    taps = sp["tap_offsets"]
                    act_set = sp["act_set"]
                    nd = len(taps)
                    live = [t for t in range(nd)
                            if not (last_gran and t in sp["tail_pe"])]
                    if not live:
                        continue  # all taps on the PE for the tail SBs

                    def span(t_idx):
                        dy, dx = taps[t_idx]
                        off0 = (PAD + r0 + dy) * WP + dx
                        return xpb_sb[:, off0: off0 + Y_ROWS * WP]

                    def center(t_idx):
                        dy, dx = taps[t_idx]
                        return xb3[:, PAD + r0 + dy: PAD + r0 + dy + Y_ROWS,
                                   PAD + dx: PAD + dx + W]

                    def kcol(t_idx):
                        cb = ks_base[oi] + t_idx
                        return ks_sb[:, cb:cb + 1]

                    # deep y rotation lets the tap engines start a granule as
                    # soon as possible -- the y-buffer WAR release (not DVE
                    # total work) gates the pipeline tail
                    y = yp.tile([C, Y_ROWS * WP], bf16, tag=f"y{g}",
                                bufs=6 if g == 4 else 4)
                    y3 = y.rearrange("p (r w) -> p r w", w=WP)
                    yc = y3[:, :, PAD:PAD + W]
                    u = u3 = None
                    if any(t not in act_set for t in live):
                        u = up.tile([C, Y_ROWS * WP], bf16, tag=f"u{g}",
                                    bufs=3)
                        u3 = u.rearrange("p (r w) -> p r w", w=WP)
                    uas, na = [], 0
                    if act_set:
                        # ACT products cover only the consumed 128-col
                        # center (ACT has no perf modes -- cost is element
                        # count; the DVE muls keep full contiguous spans
                        # for 4x mode)
                        for j in range(2):
                            ua = up.tile([C, Y_ROWS * W], bf16,
                                         tag=f"ua{g}{j}", bufs=2)
                            uas.append(
                                (ua, ua.rearrange("p (r w) -> p r w", w=W)))

                    t0 = live[0]
                    if t0 in act_set:
                        # center-only init: y's pad columns are never read
                        nc.scalar.activation(
                            yc, center(t0),
                            mybir.ActivationFunctionType.Copy,
                            scale=kcol(t0))
                    else:
                        nc.vector.tensor_scalar_mul(y, span(t0), kcol(t0))
                    for t in live[1:]:
                        if t in act_set:
                            flat, v3 = uas[na % 2]
                            na += 1
                            nc.scalar.activation(
                                flat, center(t),
                                mybir.ActivationFunctionType.Copy,
                                scale=kcol(t))
                            src = v3
                        else:
                            nc.vector.tensor_scalar_mul(u, span(t), kcol(t))
                            src = u3[:, :, PAD:PAD + W]
                        nc.vector.tensor_tensor(
                            yc, yc, src, mybir.AluOpType.add)
                    pair_ys.append((y3, PW_BLK + oi))
                return pair_ys

            granule_ys = {0: emit_taps(0)}
            for s in range(N_SB):
                r0 = s * SB_ROWS
                half = (s % SB_PER_G) * SB_ROWS
                ys = [(y3[:, half:half + SB_ROWS, PAD:PAD + W], blk)
                      for y3, blk in granule_ys[s // SB_PER_G]]

                psum = psum_pool.tile([C, SB_ROWS * W], f32, tag="acc")
                pair_list = [(offsets[2 * p], offsets[2 * p + 1], p)
                             for p in range(n_pairs)]
                if s // SB_PER_G == N_SB // SB_PER_G - 1:
                    pair_list += [
                        (extra_offsets[2 * j], extra_offsets[2 * j + 1],
                         n_pairs + j) for j in range(n_extra // 2)]
                for i, (o_a, o_b, p) in enumerate(pair_list):
                    for u_ in range(n_sub):
                        a0 = PAD + r0 + u_ * SUB_ROWS
                        nc.tensor.matmul(
                            psum[:, u_ * SUB_ROWS * W:(u_ + 1) * SUB_ROWS * W],
                            wblk8_pair(p), pair_rhs(o_a, o_b, a0, SUB_ROWS),
                            start=(i == 0), stop=False,
                            perf_mode=mybir.MatmulPerfMode.DoubleRow)
                if n_single:
                    di, dj = offsets[-1]
                    for u_ in range(n_sub):
                        a0 = PAD + r0 + u_ * SUB_ROWS + di
                        nc.tensor.matmul(
                            psum[:, u_ * SUB_ROWS * W:(u_ + 1) * SUB_ROWS * W],
                            wblk8(n_terms - 1),
                            xp3[:, a0: a0 + SUB_ROWS, PAD + dj: PAD + dj + W],
                            start=False, stop=False)
                for yv, blk in ys:
                    for u_ in range(n_sub):
                        nc.tensor.matmul(
                            psum[:, u_ * SUB_ROWS * W:(u_ + 1) * SUB_ROWS * W],
                            wblkb(blk),
                            yv[:, u_ * SUB_ROWS:(u_ + 1) * SUB_ROWS, :],
                            start=False, stop=False)

                if s == 0:
                    emit_corrections()
                # prefetch next granule's taps ahead of this SB's
                # evacuation so the DVE/ACT/GpSimd queues never sit behind
                # the psum drain
                if s % SB_PER_G == 0 and s // SB_PER_G + 1 < N_SB // SB_PER_G:
                    granule_ys[s // SB_PER_G + 1] = emit_taps(s // SB_PER_G + 1)

                # fold strip corrections into PSUM on the PE: identity-weight
                # matmuls add corr_sb rows into strided psum positions
                psum3 = psum.rearrange("p (r w) -> p r w", w=W)
                # all 4 col strips in ONE ident matmul: their psum columns
                # {7,11,116,120} factor as c0 + a*da + b*db with the strips
                # laid out j = 2a+b in corr_sb -> congruent 4-dim APs
                cols = [st["fixed_out"] for st in strips if st["kind"] == "col"]
                c0, db, da = cols[0], cols[1] - cols[0], cols[2] - cols[0]
                assert cols == [c0, c0 + db, c0 + da, c0 + da + db]
                src0 = corr_sb[:, r0: r0 + SB_ROWS]
                rhs4 = AP(tensor=src0.tensor, offset=src0.offset,
                          ap=[src0.ap[0], (2 * H, 2), (H, 2), (1, SB_ROWS)])
                dst0 = psum[:, c0:c0 + 1]
                out4 = AP(tensor=dst0.tensor, offset=dst0.offset,
                          ap=[dst0.ap[0], (da, 2), (db, 2), (W, SB_ROWS)])
                strip_mms = [(out4, rhs4)]
                for j, st in enumerate(strips):
                    if st["kind"] == "row" and r0 <= st["fixed_out"] < r0 + SB_ROWS:
                        lr = st["fixed_out"] - r0
                        strip_mms.append((psum3[:, lr:lr + 1, :],
                                          corr_sb[:, j * H: j * H + W]))
                for i, (dst, src) in enumerate(strip_mms):
                    nc.tensor.matmul(dst, wblkb(ID_BLK), src,
                                     start=False, stop=False)
                # residual into PSUM: ident*WSCALE applied to the bf16 image
                # (0.4% of |x|, inside the error budget; saves the fp32 x DMA
                # and keeps the whole evacuation off the busy Vector engine)
                for u_ in range(n_sub):
                    a0 = PAD + r0 + u_ * SUB_ROWS
                    nc.tensor.matmul(
                        psum[:, u_ * SUB_ROWS * W:(u_ + 1) * SUB_ROWS * W],
                        wblkb(RES_BLK),
                        xb3[:, a0: a0 + SUB_ROWS, PAD:PAD + W],
                        start=False, stop=(u_ == n_sub - 1))

                # single DVE dual-op: out = (psum + beff*4096) * 2^-12.
                # (ACT would need two ops -- HW drops the scale when scale
                # and bias are combined -- and the DVE has the spare
                # capacity now that ACT carries most tap multiplies.)
                out_sb = outp.tile([C, SB_ROWS * W], f32)
                nc.vector.tensor_scalar(out_sb, psum, beff_sb[:, 0:1],
                                        1.0 / WSCALE,
                                        mybir.AluOpType.add,
                                        mybir.AluOpType.mult)
                nc.gpsimd.dma_start(out=out_d[:, r0 * W:(r0 + SB_ROWS) * W],
                                    in_=out_sb)
    nc.finalize()
    return nc


def _make_in_maps(inputs):
    x = np.ascontiguousarray(inputs["x"], dtype=np.float32)
    (wt8, wtb, ks, beff, offsets, off_specs, strips,
     extra_offsets) = _build_weights(inputs)
    if "nc" not in _CACHE:
        _CACHE["nc"] = _build_program(offsets, off_specs, strips,
                                      extra_offsets)

    import ml_dtypes
    bf = ml_dtypes.bfloat16
    f8 = ml_dtypes.float8_e4m3
    xpad8 = np.zeros((B, C, HP, WP), f8)
    xpad8[:, :, PAD:PAD + H, PAD:PAD + W] = x.astype(f8)
    xpadb = np.zeros((B, C, HP, WP), bf)
    xpadb[:, :, PAD:PAD + H, PAD:PAD + W] = x.astype(bf)
    beff_col = np.ascontiguousarray((beff * WSCALE).reshape(C, 1))
    wt8_f8 = wt8.astype(f8)
    wtb_bf = wtb.astype(bf)
    ksc = np.ascontiguousarray(ks)
    return [
        {
            "xp8": np.ascontiguousarray(xpad8[b].reshape(C, HP * WP)),
            "xpb": np.ascontiguousarray(xpadb[b].reshape(C, HP * WP)),
            "xbrd": np.ascontiguousarray(np.concatenate(
                [xpadb[b, :, :, PAD + 0], xpadb[b, :, :, PAD + W - 1],
                 xpadb[b, :, PAD + 0, :], xpadb[b, :, PAD + H - 1, :]],
                axis=1)),
            "wt8": wt8_f8,
            "wtb": wtb_bf,
            "ks": ksc,
            "beff": beff_col,
        }
        for b in range(B)
    ]


def kernel(**inputs):
    in_maps = _make_in_maps(inputs)
    from concourse.bass_utils import run_bass_kernel_spmd
    res = run_bass_kernel_spmd(_CACHE["nc"], in_maps, core_ids=list(range(N_CORES)))
    out = np.stack([res.results[b]["out"].reshape(C, H, W) for b in range(B)])
    return out.astype(np.float32)

